# revision 1
# baseline (speedup 1.0000x reference)
"""Trainium2 Bass kernel for nn_CodeExpressionContextMixer.

Computes, for a mapping (key -> val) over AST/CFG node tables:
    u   = tanh(cfg[val] @ W_update + b_update)
    z   = sigmoid(prev[key] @ Wg1 + u @ Wg2 + b_gate)
    out = prev.at[key].set(z * prev[key] + (1 - z) * u)

Strategy (8 NeuronCores, SPMD, no collectives):
  * Dense formulation: 80% of rows are mapped, so every row is processed.
    Host scatters (val, mask) into dense per-row arrays; rows without a
    mapping get mask "off" and their update is numerically zeroed, so the
    kernel is a single sequential pass with a gather (no scatter).
  * prev is row-sharded across the 8 cores and passed TRANSPOSED
    (feature-major) so the per-tile matmuls need no transposes of prev and
    the output is written back transposed (host untransposes).
  * cfg is replicated; ctx rows are fetched with indirect (gather) DMAs and
    transposed on the PE.
  * Gate weights/bias are negated on host so the ACT computes
    zp = 1 - z = sigmoid(-(arg)); unmapped rows ride a -30000 additive
    mask-bias row folded into the gate matmul PSUM group => zp == 0 =>
    out = prev exactly.
  * Matmuls run as float32r (full PE rate for 512-wide moving operands).
"""

import os
import numpy as np

R = 500000          # AST rows
CFGN = 100000       # CFG rows
D = 256             # feature dim
NCORES = 8
SB = 512            # rows per superblock
KB = SB // 128      # 128-row blocks per superblock
SHARD = 62976       # padded rows per core = 512 * 123; 8*62976 = 503808 >= R
NSB = SHARD // SB   # superblocks per core
MASK_OFF = -30000.0

_cache = {}


def _build():
    """Build + compile the Bass program once per process."""
    if "nc" in _cache:
        return _cache["nc"]
    from contextlib import ExitStack
    import concourse.bass as bass
    import concourse.tile as tile
    from concourse import bacc, mybir
    from concourse.masks import make_identity

    F32 = mybir.dt.float32
    F32R = mybir.dt.float32r
    BF16 = mybir.dt.bfloat16
    I32 = mybir.dt.int32
    AF = mybir.ActivationFunctionType

    nc = bacc.Bacc("TRN2", target_bir_lowering=False, debug=False)

    prevT = nc.dram_tensor("prevT", [D, SHARD], F32R, kind="ExternalInput").ap()
    cfg = nc.dram_tensor("cfg", [CFGN, D], F32, kind="ExternalInput").ap()
    valt = nc.dram_tensor("valt", [128, SHARD // 128], I32, kind="ExternalInput").ap()
    mrow = nc.dram_tensor("mrow", [1, SHARD], BF16, kind="ExternalInput").ap()
    wu = nc.dram_tensor("wu", [D, D], F32R, kind="ExternalInput").ap()
    wgn = nc.dram_tensor("wgn", [2 * D, D], F32R, kind="ExternalInput").ap()
    bu = nc.dram_tensor("bu", [128, D // 128], F32, kind="ExternalInput").ap()
    bgn = nc.dram_tensor("bgn", [128, D // 128], F32, kind="ExternalInput").ap()
    outT = nc.dram_tensor("outT", [D, SHARD], F32, kind="ExternalOutput").ap()

    es = ExitStack()
    with tile.TileContext(nc) as tc:
        cpool = es.enter_context(tc.tile_pool(name="const", bufs=1))
        pool = es.enter_context(tc.tile_pool(name="sbuf", bufs=3))
        psum = es.enter_context(tc.tile_pool(name="psum", bufs=1, space="PSUM"))

        ident = cpool.tile([128, 128], F32)
        make_identity(nc, ident[:])
        ones_row = cpool.tile([1, 128], BF16)
        nc.vector.memset(ones_row[:], 1.0)

        wu_sb = []
        for k in range(2):
            t = cpool.tile([128, D], F32R, tag=f"wu{k}")
            nc.sync.dma_start(t[:], wu[128 * k : 128 * (k + 1), :])
            wu_sb.append(t)
        wgn_sb = []
        for k in range(4):
            t = cpool.tile([128, D], F32R, tag=f"wgn{k}")
            nc.sync.dma_start(t[:], wgn[128 * k : 128 * (k + 1), :])
            wgn_sb.append(t)
        bu_sb = cpool.tile([128, D // 128], F32)
        nc.sync.dma_start(bu_sb[:], bu[:])
        bgn_sb = cpool.tile([128, D // 128], F32)
        nc.sync.dma_start(bgn_sb[:], bgn[:])
        valt_sb = cpool.tile([128, SHARD // 128], I32)
        nc.sync.dma_start(valt_sb[:], valt[:])

        for s in range(NSB):
            rb = s * SB
            # prev (transposed) k-chunks: [feat 128k:128k+128, rows rb:rb+SB]
            PT = []
            for k in range(2):
                t = pool.tile([128, SB], F32R, tag=f"pt{k}")
                nc.sync.dma_start(t[:], prevT[128 * k : 128 * (k + 1), rb : rb + SB])
                PT.append(t)
            # mask-bias row for this superblock
            mr = pool.tile([1, SB], BF16, tag="mr")
            nc.sync.dma_start(mr[:], mrow[:, rb : rb + SB])
            # gather ctx rows, one indirect DMA per 128-row block
            C = pool.tile([128, KB, D], F32, tag="c")
            for j in range(KB):
                nc.gpsimd.indirect_dma_start(
                    out=C[:, j, :],
                    out_offset=None,
                    in_=cfg[:],
                    in_offset=bass.IndirectOffsetOnAxis(
                        ap=valt_sb[:, s * KB + j : s * KB + j + 1], axis=0
                    ),
                )
            # transpose ctx into feature-major k-chunks
            CT = []
            for k in range(2):
                ctps = psum.tile([128, SB], F32, tag=f"ctps{k}")
                for j in range(KB):
                    nc.tensor.transpose(
                        out=ctps[:, 128 * j : 128 * (j + 1)],
                        in_=C[:, j, 128 * k : 128 * (k + 1)],
                        identity=ident[:],
                    )
                t = pool.tile([128, SB], F32R, tag=f"ct{k}")
                nc.scalar.copy(t[:], ctps[:])
                CT.append(t)
            # uT[m] = tanh(sum_k Wu[k,m].T @ CT[k] + bu[m])
            UT = []
            for m in range(2):
                ups = psum.tile([128, SB], F32, tag=f"ups{m}")
                for k in range(2):
                    nc.tensor.matmul(
                        out=ups[:],
                        lhsT=wu_sb[k][:, 128 * m : 128 * (m + 1)],
                        rhs=CT[k][:],
                        start=(k == 0),
                        stop=(k == 1),
                    )
                t = pool.tile([128, SB], F32R, tag=f"ut{m}")
                nc.scalar.activation(t[:], ups[:], AF.Tanh, bias=bu_sb[:, m : m + 1])
                UT.append(t)
            # zp[m] = sigmoid(-(p@Wg1 + u@Wg2 + bg) + maskbias)
            ZP = []
            for m in range(2):
                zps = psum.tile([128, SB], F32, tag=f"zps{m}")
                for k in range(2):
                    nc.tensor.matmul(
                        out=zps[:],
                        lhsT=wgn_sb[k][:, 128 * m : 128 * (m + 1)],
                        rhs=PT[k][:],
                        start=(k == 0),
                        stop=False,
                    )
                for k in range(2):
                    nc.tensor.matmul(
                        out=zps[:],
                        lhsT=wgn_sb[2 + k][:, 128 * m : 128 * (m + 1)],
                        rhs=UT[k][:],
                        start=False,
                        stop=False,
                    )
                nc.tensor.matmul(
                    out=zps[:], lhsT=ones_row[:], rhs=mr[:], start=False, stop=True
                )
                t = pool.tile([128, SB], F32, tag=f"zp{m}")
                nc.scalar.activation(t[:], zps[:], AF.Sigmoid, bias=bgn_sb[:, m : m + 1])
                ZP.append(t)
            # out = p + zp * (u - p), per feature k-chunk
            for k in range(2):
                tdif = pool.tile([128, SB], F32, tag=f"td{k}")
                nc.vector.tensor_sub(tdif[:], UT[k][:].bitcast(F32), PT[k][:].bitcast(F32))
                nc.vector.tensor_mul(tdif[:], tdif[:], ZP[k][:])
                o = pool.tile([128, SB], F32, tag=f"o{k}")
                nc.vector.tensor_add(o[:], PT[k][:].bitcast(F32), tdif[:])
                nc.sync.dma_start(outT[128 * k : 128 * (k + 1), rb : rb + SB], o[:])
        es.close()
    nc.compile()
    _cache["nc"] = nc
    return nc


def _prep_inputs(prev, cfg, map_key, map_val, W_update, b_update, W_gate, b_gate):
    """Host-side shard prep. Returns per-core input maps."""
    prev = np.ascontiguousarray(prev, dtype=np.float32)
    cfg = np.ascontiguousarray(cfg, dtype=np.float32)
    import ml_dtypes

    total = NCORES * SHARD
    val_dense = np.zeros(total, np.int32)
    maskb = np.full(total, MASK_OFF, np.float32)
    val_dense[map_key] = map_val
    maskb[map_key] = 0.0

    wu = np.ascontiguousarray(W_update, dtype=np.float32)
    wgn = np.ascontiguousarray(-W_gate, dtype=np.float32)
    bu2 = np.ascontiguousarray(b_update.reshape(2, 128).T, dtype=np.float32)
    bgn2 = np.ascontiguousarray((-b_gate).reshape(2, 128).T, dtype=np.float32)

    in_maps = []
    for c in range(NCORES):
        r0 = c * SHARD
        r1 = min(r0 + SHARD, R)
        n = r1 - r0
        pT = np.zeros((D, SHARD), np.float32)
        pT[:, :n] = prev[r0:r1].T
        vt = np.ascontiguousarray(
            val_dense[r0 : r0 + SHARD].reshape(SHARD // 128, 128).T
        )
        mr = maskb[r0 : r0 + SHARD][None, :].astype(ml_dtypes.bfloat16)
        in_maps.append(
            {
                "prevT": pT,
                "cfg": cfg,
                "valt": vt,
                "mrow": mr,
                "wu": wu,
                "wgn": wgn,
                "bu": bu2,
                "bgn": bgn2,
            }
        )
    return in_maps


def _run(in_maps, profile_dir=None):
    from concourse import bass_utils

    nc = _build()
    if profile_dir is None:
        res = bass_utils.run_bass_kernel_spmd(nc, in_maps, core_ids=list(range(NCORES)))
        return res.results
    # profiled run: capture NTFFs around the PJRT execute
    from trn_agent_boot.trn_boot import _ntff_profile_via_ctypes

    hook = _ntff_profile_via_ctypes("/opt/axon/libaxon_pjrt.so")
    os.makedirs(profile_dir, exist_ok=True)
    with hook(profile_dir, list(range(NCORES))):
        res = bass_utils.run_bass_kernel_spmd(nc, in_maps, core_ids=list(range(NCORES)))
    return res.results


def _unshard(results):
    out = np.empty((R, D), np.float32)
    for c in range(NCORES):
        r0 = c * SHARD
        r1 = min(r0 + SHARD, R)
        out[r0:r1] = results[c]["outT"][:, : r1 - r0].T
    return out


def kernel(
    previous_ast_nodes_encodings,
    new_cfg_nodes_encodings,
    map_key_indices,
    map_val_indices,
    W_update,
    b_update,
    W_gate,
    b_gate,
):
    in_maps = _prep_inputs(
        np.asarray(previous_ast_nodes_encodings),
        np.asarray(new_cfg_nodes_encodings),
        np.asarray(map_key_indices),
        np.asarray(map_val_indices),
        np.asarray(W_update),
        np.asarray(b_update),
        np.asarray(W_gate),
        np.asarray(b_gate),
    )
    results = _run(in_maps, profile_dir=os.environ.get("KERNEL_PROFILE_DIR") or None)
    return _unshard(results)
